# revision 12
# baseline (speedup 1.0000x reference)
"""MemN2N Bass kernel (SPMD over 8 cores), v2.

Strategy vs v0 baseline (3.96 ms):
  - The 4 embedding tables are interleaved host-side into one bf16 table
    embI [V+1, 4*E]; each story token is ONE 1KB gather descriptor covering
    all 4 tables (v0: 4 separate f32 gathers -> 4x bytes, 50x instructions).
  - Gathers are batched: one indirect DMA per 25-token chunk (3200
    descriptors) instead of per-token-column (128 descriptors). This needs a
    larger SWDGE descriptor carveout (dynamic_dma_scratch_size).
  - The question is packed as an extra "sentence" (s = S) of every batch, so
    its embedding-bag rides the story gather for free; u0 is extracted with
    one PE transpose.
  - Attention hops run entirely on-chip (no DRAM bounces): scores come from
    a DVE multiply+reduce against a partition-replicated u, softmax
    normalization uses tiny bmask matmuls for the cross-partition sums.
  - Final vocab softmax is vocab-parallel: u3 is AllGather'd (32KB), each
    core computes logits/exp for ALL 64 batches over its 12.8k-vocab slice
    (PE: 100 [128x128]@[128x64] matmuls), the softmax denominator is
    AllReduce'd ([1,64]), and each core writes its [12800, 64] output slice
    which the host transposes/concats.

Per-core HBM traffic ~90 MB (vs ~196 MB), instruction count ~300 (vs 2800).
"""
import sys

sys.path.insert(0, "/opt/trn_rl_repo")

from contextlib import ExitStack

import numpy as np

import concourse.bass as bass
import concourse.mybir as mybir
import concourse.tile as tile
from concourse.masks import make_identity

F32 = mybir.dt.float32
BF16 = mybir.dt.bfloat16
I32 = mybir.dt.int32
AX = mybir.AxisListType
ALU = mybir.AluOpType
ACTF = mybir.ActivationFunctionType

P = 128
E = 128
N_CORES = 8


class Cfg:
    def __init__(self, B=64, S=200, SENT=50, V=100000, K_HOP=3, TCH=25, CHT=8):
        self.B = B
        self.B_LOC = B // N_CORES  # 8
        self.S = S  # 200 real sentences
        self.SENT = SENT  # 50 tokens per sentence
        self.V = V
        self.K_HOP = K_HOP
        self.NT = K_HOP + 1  # 4 tables
        self.EI = self.NT * E  # 512 interleaved embedding width
        self.PPB = P // self.B_LOC  # 16 partitions per batch
        # sentence slots per batch padded to use all 128 partitions; S+1
        # sentences (the question rides as sentence index S)
        self.SPP = -(-(self.B_LOC * (S + 1)) // P)  # 13
        self.S_PAD = self.PPB * self.SPP  # 208
        assert self.S_PAD >= S + 1
        self.TOT = P * self.SPP  # 1664 slots
        # question slot: sentence s=S lives at (q = S//SPP, j = S%SPP)
        self.QJ = S % self.SPP
        # token gather chunking
        self.TCH = TCH  # tokens per indirect DMA
        self.NTOK = self.SPP * SENT  # idx columns per partition (650)
        assert self.NTOK % TCH == 0
        self.NCHG = self.NTOK // TCH  # 26 gather chunks
        assert SENT % TCH == 0 or TCH % SENT == 0 or SENT % TCH == 0
        self.CPS = SENT // TCH  # chunks per sentence (2)
        assert self.CPS * TCH == SENT
        # vocab-parallel final phase
        self.VPAD = -(-V // (N_CORES * P)) * (N_CORES * P)  # 102400
        self.VS = self.VPAD // N_CORES  # 12800 vocab rows per core
        self.NVT = self.VS // P  # 100 v-tiles per core
        self.CHT = CHT  # v-tiles per psum chunk
        self.NCH = -(-self.NVT // CHT)  # 13 chunks
        # [128, 1] offsets -> 128 descriptors per gather
        self.DMA_SCRATCH = 32768
        self.NQ = 4  # SWDGE queues to spread gathers over


def build_kernel(cfg: Cfg, nc: bass.Bass, dbg: bool = False):
    c = cfg
    story = nc.declare_dram_parameter("story_pad", [c.TOT, c.SENT], I32, isOutput=False)
    embI = nc.declare_dram_parameter("embI", [c.V + 1, c.EI], BF16, isOutput=False)
    emb3s = nc.declare_dram_parameter("emb3s", [E, c.VS], BF16, isOutput=False)
    vmask = nc.declare_dram_parameter("vmask", [P, c.NVT], F32, isOutput=False)
    smask = nc.declare_dram_parameter("smask", [P, c.SPP], F32, isOutput=False)
    bmask = nc.declare_dram_parameter("bmask", [P, c.B_LOC], F32, isOutput=False)
    bmaskT = nc.declare_dram_parameter("bmaskT", [c.B_LOC, P], F32, isOutput=False)
    out = nc.declare_dram_parameter("out", [c.VS, c.B], BF16, isOutput=True)

    dbgout = None
    if dbg:
        dbgout = {
            "dG0": nc.declare_dram_parameter("dG0", [P, c.EI], F32, isOutput=True),
            "duT0": nc.declare_dram_parameter("duT0", [P, c.B_LOC], F32, isOutput=True),
            "duT1": nc.declare_dram_parameter("duT1", [P, c.B_LOC], F32, isOutput=True),
            "dprobs0": nc.declare_dram_parameter("dprobs0", [P, c.SPP], F32, isOutput=True),
            "duall": nc.declare_dram_parameter("duall", [P, c.B], F32, isOutput=True),
            "dden": nc.declare_dram_parameter("dden", [1, c.B], F32, isOutput=True),
        }
    with tile.TileContext(nc) as tc:
        _body(c, nc, tc, story, embI, emb3s, vmask, smask, bmask, bmaskT, out, dbgout)
    return nc


def _body(c: Cfg, nc, tc, story, embI, emb3s, vmask, smask, bmask, bmaskT, out, dbgout=None):
    B = c.B
    BL = c.B_LOC
    with ExitStack() as es:
        cpool = es.enter_context(tc.tile_pool(name="const", bufs=1))
        upool = es.enter_context(tc.tile_pool(name="u", bufs=1))

        identity = cpool.tile([P, P], F32)
        make_identity(nc, identity[:])

        idx_t = cpool.tile([P, c.NTOK], I32)
        nc.sync.dma_start(
            out=idx_t[:], in_=story[:].rearrange("(p j) t -> p (j t)", p=P)
        )
        smask_t = cpool.tile([P, c.SPP], F32)
        nc.sync.dma_start(out=smask_t[:], in_=smask[:])
        vmask_t = cpool.tile([P, c.NVT], F32)
        nc.sync.dma_start(out=vmask_t[:], in_=vmask[:])
        bmask_t = cpool.tile([P, BL], F32)
        nc.sync.dma_start(out=bmask_t[:], in_=bmask[:])
        bmaskT_t = cpool.tile([BL, P], F32)
        nc.sync.dma_start(out=bmaskT_t[:], in_=bmaskT[:])
        ones_col = cpool.tile([P, 1], F32)
        nc.vector.memset(ones_col[:], 1.0)
        ones_row = cpool.tile([1, P], F32)
        nc.vector.memset(ones_row[:], 1.0)

        # G_all[slot, j, t*E + e] = sum over sentence tokens of interleaved tables
        G_all = cpool.tile([P, c.SPP, c.EI], F32)

        # warmup collective: opens the CC channel while gathers run, so the
        # real AllGather at the tail doesn't pay first-use setup latency
        wdram = es.enter_context(tc.tile_pool(name="wdram", bufs=1, space="DRAM"))
        wu_in = wdram.tile([1, BL], F32, name="wu_in")
        nc.gpsimd.dma_start(out=wu_in[:], in_=bmask_t[:1, :])
        wu_out = wdram.tile([N_CORES, BL], F32, name="wu_out")
        nc.gpsimd.collective_compute(
            "AllGather",
            ALU.bypass,
            replica_groups=[list(range(N_CORES))],
            ins=[wu_in.opt()],
            outs=[wu_out.opt()],
        )
        # keep it alive: wu_out[0,0] == 1.0 == ones_col[0,0]
        nc.gpsimd.dma_start(out=ones_col[:1, :1], in_=wu_out[:1, :1])

        # ---------- phase A: gather + segment reduce ----------
        # HW ucode only honors [128, 1] offset APs (one gather instruction per
        # token column; 128 descriptors each). The per-sentence token sum is a
        # pairwise in-place bf16 tree: tensor_tensor add has a 2x DVE mode for
        # packed 2-byte operands (tensor_reduce has none). The last level
        # writes f32 into G_all.
        def tree_sum(buf, n):
            while n > 2:
                k = n // 2
                nc.vector.tensor_add(
                    out=buf[:, 0:k, :], in0=buf[:, 0:k, :], in1=buf[:, n - k : n, :]
                )
                n = k + (n & 1)

        with tc.tile_pool(name="gather", bufs=2) as gbpool:
            for j in range(c.SPP):
                gbuf = gbpool.tile([P, c.SENT, c.EI], BF16, tag="gbuf")
                for s in range(c.SENT):
                    gi = nc.gpsimd.indirect_dma_start(
                        out=gbuf[:, s, :],
                        out_offset=None,
                        in_=embI[:],
                        in_offset=bass.IndirectOffsetOnAxis(
                            ap=idx_t[:, j * c.SENT + s : j * c.SENT + s + 1], axis=0
                        ),
                    )
                    # spread SWDGE descriptor generation across queues
                    q = s % c.NQ
                    if q:
                        gi.ins.queue = f"qPoolDynamic{q}"
                tree_sum(gbuf, c.SENT)
                nc.vector.tensor_add(
                    out=G_all[:, j, :], in0=gbuf[:, 0, :], in1=gbuf[:, 1, :]
                )

        if dbgout is not None:
            nc.sync.dma_start(out=dbgout["dG0"][:], in_=G_all[:, 0, :])

        # ---------- final-phase weights prefetch (overlaps hops) ----------
        fpool = es.enter_context(tc.tile_pool(name="final", bufs=1))
        emb3s_t = fpool.tile([P, c.VS], BF16)
        nc.sync.dma_start(out=emb3s_t[:], in_=emb3s[:])

        # ---------- phase B: u0 from the question slot ----------
        uT_sb = None
        with tc.tile_pool(name="tp_ps", bufs=2, space="PSUM") as tppool:
            t0_ps = tppool.tile([P, P], F32, tag="t0")
            nc.tensor.matmul(
                out=t0_ps[:], lhsT=G_all[:, c.QJ, 0:E], rhs=identity[:],
                start=True, stop=True,
            )
            uT_sb = upool.tile([P, BL], F32, tag="uT0")
            # question of batch b sits at slot partition b*PPB + (PPB-1)
            nc.vector.tensor_copy(
                out=uT_sb[:],
                in_=t0_ps[:].rearrange("e (b q) -> e b q", q=c.PPB)[:, :, c.PPB - 1],
            )
            if dbgout is not None:
                nc.sync.dma_start(out=dbgout["duT0"][:], in_=uT_sb[:])

        # ---------- phase C: K_HOP attention hops, fully on-chip ----------
        with (
            tc.tile_pool(name="hop", bufs=2) as hpool,
            tc.tile_pool(name="hop_ps", bufs=1, space="PSUM") as hps,
            tc.tile_pool(name="hop_ps2", bufs=1, space="PSUM") as hps2,
        ):
            for h in range(c.K_HOP):
                # u_b [BL, E] = uT^T
                ub_ps = hps.tile([BL, P], F32, tag="ub")
                nc.tensor.matmul(
                    out=ub_ps[:], lhsT=uT_sb[:], rhs=identity[:], start=True, stop=True
                )
                ub_sb = hpool.tile([BL, P], F32, tag="ub_sb")
                nc.vector.tensor_copy(out=ub_sb[:], in_=ub_ps[:])
                # urep [p, e] = u[batch(p), e]
                urep_ps = hps.tile([P, P], F32, tag="urep")
                nc.tensor.matmul(
                    out=urep_ps[:], lhsT=bmaskT_t[:], rhs=ub_sb[:], start=True, stop=True
                )
                # scores[p, j] = sum_e G_m[p, j, e] * urep[p, e] (read PSUM)
                sc_tmp = hpool.tile([P, c.SPP, E], F32, tag="sc_tmp")
                nc.vector.tensor_tensor(
                    out=sc_tmp[:],
                    in0=G_all[:, :, h * E : (h + 1) * E],
                    in1=urep_ps[:].unsqueeze(1).to_broadcast([P, c.SPP, E]),
                    op=ALU.mult,
                )
                scores = hpool.tile([P, c.SPP], F32, tag="scores")
                nc.vector.tensor_reduce(
                    out=scores[:].unsqueeze(-1), in_=sc_tmp[:], axis=AX.X, op=ALU.add
                )
                # masked exp (scores are O(10); no max-subtraction needed in f32)
                exps = hpool.tile([P, c.SPP], F32, tag="exps")
                nc.scalar.activation(out=exps[:], in_=scores[:], func=ACTF.Exp)
                nc.vector.tensor_tensor(
                    out=exps[:], in0=exps[:], in1=smask_t[:], op=ALU.mult
                )
                rowsum = hpool.tile([P, 1], F32, tag="rowsum")
                nc.vector.tensor_reduce(
                    out=rowsum[:], in_=exps[:], axis=AX.X, op=ALU.add
                )
                den_ps = hps2.tile([BL, 1], F32, tag="den")
                nc.tensor.matmul(
                    out=den_ps[:], lhsT=bmask_t[:], rhs=rowsum[:], start=True, stop=True
                )
                rec8 = hpool.tile([BL, 1], F32, tag="rec8")
                nc.vector.reciprocal(out=rec8[:], in_=den_ps[:])
                rep_ps = hps2.tile([P, 1], F32, tag="rep")
                nc.tensor.matmul(
                    out=rep_ps[:], lhsT=bmaskT_t[:], rhs=rec8[:], start=True, stop=True
                )
                probs = hpool.tile([P, c.SPP], F32, tag="probs")
                nc.vector.tensor_scalar_mul(probs[:], exps[:], rep_ps[:])
                if dbgout is not None and h == 0:
                    nc.sync.dma_start(out=dbgout["dprobs0"][:], in_=probs[:])
                # block-diagonal weights and combine
                bd = hpool.tile([P, c.SPP, BL], F32, tag="bd")
                nc.vector.tensor_tensor(
                    out=bd[:],
                    in0=probs[:].unsqueeze(-1).to_broadcast([P, c.SPP, BL]),
                    in1=bmask_t[:].unsqueeze(1).to_broadcast([P, c.SPP, BL]),
                    op=ALU.mult,
                )
                uc_ps = hps.tile([P, BL], F32, tag="uc")
                for j in range(c.SPP):
                    nc.tensor.matmul(
                        out=uc_ps[:],
                        lhsT=G_all[:, j, (h + 1) * E : (h + 2) * E],
                        rhs=bd[:, j, :],
                        start=(j == 0),
                        stop=(j == c.SPP - 1),
                    )
                uT_new = upool.tile([P, BL], F32, tag=f"uT{h + 1}")
                nc.vector.tensor_add(out=uT_new[:], in0=uc_ps[:], in1=uT_sb[:])
                uT_sb = uT_new
                if dbgout is not None and h == 0:
                    nc.sync.dma_start(out=dbgout["duT1"][:], in_=uT_sb[:])

        # ---------- phase D: vocab-parallel logits + softmax ----------
        with (
            tc.tile_pool(name="fin_ps", bufs=2, space="PSUM") as fps,
            tc.tile_pool(name="fin_ps2", bufs=1, space="PSUM") as fps2,
            tc.tile_pool(name="dram", bufs=1, space="DRAM") as dram,
        ):
            # AllGather u3 across the 8 cores
            u3b_ps = fps2.tile([BL, P], F32, tag="u3b")
            nc.tensor.matmul(
                out=u3b_ps[:], lhsT=uT_sb[:], rhs=identity[:], start=True, stop=True
            )
            u3b_sb = fpool.tile([BL, P], F32)
            nc.vector.tensor_copy(out=u3b_sb[:], in_=u3b_ps[:])
            uin_d = dram.tile([BL, P], F32, name="uin_d")
            nc.gpsimd.dma_start(out=uin_d[:], in_=u3b_sb[:])
            uall_d = dram.tile([B, P], F32, name="uall_d")
            nc.gpsimd.collective_compute(
                "AllGather",
                ALU.bypass,
                replica_groups=[list(range(N_CORES))],
                ins=[uin_d.opt()],
                outs=[uall_d.opt()],
            )
            uall_sb = fpool.tile([B, P], F32)
            nc.gpsimd.dma_start(out=uall_sb[:], in_=uall_d[:])
            uTall_ps = fps2.tile([P, B], F32, tag="uTall")
            nc.tensor.matmul(
                out=uTall_ps[:], lhsT=uall_sb[:], rhs=identity[:B, :B],
                start=True, stop=True,
            )
            uTall_bf = fpool.tile([P, B], BF16)
            nc.vector.tensor_copy(out=uTall_bf[:], in_=uTall_ps[:])
            if dbgout is not None:
                nc.sync.dma_start(out=dbgout["duall"][:], in_=uTall_ps[:])

            # logits + exp over this core's vocab slice, all 64 batches
            exp_buf = fpool.tile([P, c.NVT * B], F32)
            for ch in range(c.NCH):
                t0 = ch * c.CHT
                nt = min(c.CHT, c.NVT - t0)
                lg = fps.tile([P, c.CHT * B], F32, tag="lg")
                for m in range(nt):
                    nc.tensor.matmul(
                        out=lg[:, m * B : (m + 1) * B],
                        lhsT=emb3s_t[:, (t0 + m) * P : (t0 + m + 1) * P],
                        rhs=uTall_bf[:],
                        start=True,
                        stop=True,
                    )
                nc.scalar.activation(
                    out=exp_buf[:, t0 * B : t0 * B + nt * B],
                    in_=lg[:, : nt * B],
                    func=ACTF.Exp,
                )
            exp3 = exp_buf[:].rearrange("p (m b) -> p m b", b=B)
            # zero out vocab-pad rows
            nc.vector.tensor_tensor(
                out=exp3,
                in0=exp3,
                in1=vmask_t[:].unsqueeze(-1).to_broadcast([P, c.NVT, B]),
                op=ALU.mult,
            )
            # local denominator: reduce over tiles, then over partitions
            part_den = fpool.tile([P, B], F32)
            nc.vector.tensor_reduce(
                out=part_den[:].unsqueeze(-1),
                in_=exp_buf[:].rearrange("p (m b) -> p b m", b=B),
                axis=AX.X,
                op=ALU.add,
            )
            dsum_ps = fps2.tile([1, B], F32, tag="dsum")
            nc.tensor.matmul(
                out=dsum_ps[:], lhsT=ones_col[:], rhs=part_den[:], start=True, stop=True
            )
            den_sb = fpool.tile([1, B], F32)
            nc.vector.tensor_copy(out=den_sb[:], in_=dsum_ps[:])
            den_i = dram.tile([1, B], F32, name="den_i")
            nc.gpsimd.dma_start(out=den_i[:], in_=den_sb[:])
            den_o = dram.tile([1, B], F32, name="den_o")
            nc.gpsimd.collective_compute(
                "AllReduce",
                ALU.add,
                replica_groups=[list(range(N_CORES))],
                ins=[den_i.opt()],
                outs=[den_o.opt()],
            )
            dent_sb = fpool.tile([1, B], F32)
            nc.gpsimd.dma_start(out=dent_sb[:], in_=den_o[:])
            if dbgout is not None:
                nc.sync.dma_start(out=dbgout["dden"][:], in_=dent_sb[:])
            rec1 = fpool.tile([1, B], F32)
            nc.vector.reciprocal(out=rec1[:], in_=dent_sb[:])
            rep_ps = fps2.tile([P, B], F32, tag="rrep")
            nc.tensor.matmul(
                out=rep_ps[:], lhsT=ones_row[:], rhs=rec1[:], start=True, stop=True
            )
            rec_sb = fpool.tile([P, B], F32)
            nc.vector.tensor_copy(out=rec_sb[:], in_=rep_ps[:])
            prob_bf = fpool.tile([P, c.NVT * B], BF16)
            nc.vector.tensor_tensor(
                out=prob_bf[:].rearrange("p (m b) -> p m b", b=B),
                in0=exp3,
                in1=rec_sb[:].unsqueeze(1).to_broadcast([P, c.NVT, B]),
                op=ALU.mult,
            )
            # out[v = m*128 + p, b] = probs
            nc.sync.dma_start(
                out=out[:].rearrange("(m p) b -> p m b", p=P),
                in_=prob_bf[:].rearrange("p (m b) -> p m b", b=B),
            )


# ---------------- host-side pack/unpack ----------------
_CACHE = {}


def _get_nc(cfg):
    key = "nc"
    if key not in _CACHE:
        import concourse.bacc as bacc

        nc = bacc.Bacc(
            target_bir_lowering=False,
            dynamic_dma_scratch_size=cfg.DMA_SCRATCH,
            num_swdge_queues=cfg.NQ,
        )
        build_kernel(cfg, nc)
        nc.finalize()
        _CACHE[key] = nc
    return _CACHE[key]


def _pack_shared(cfg, emb_A):
    key = "shared"
    if key not in _CACHE or _CACHE[key][0] is not emb_A:
        c = cfg
        import ml_dtypes

        NT, V, _ = emb_A.shape
        # interleaved bf16 table, zero pad row at V
        embI = np.zeros((V + 1, c.EI), ml_dtypes.bfloat16)
        embI[:V] = emb_A.transpose(1, 0, 2).reshape(V, c.EI)
        # per-core transposed slices of the last table
        e3T = np.zeros((E, c.VPAD), np.float32)
        e3T[:, :V] = emb_A[NT - 1].T
        e3T = e3T.astype(ml_dtypes.bfloat16)
        emb3s_all = [
            np.ascontiguousarray(e3T[:, ci * c.VS : (ci + 1) * c.VS])
            for ci in range(N_CORES)
        ]
        vmask_all = []
        for ci in range(N_CORES):
            vrow = ci * c.VS + np.arange(c.NVT)[None, :] * P + np.arange(P)[:, None]
            vmask_all.append((vrow < V).astype(np.float32))
        pj = np.arange(P)[:, None] % c.PPB
        jj = np.arange(c.SPP)[None, :]
        smask = (pj * c.SPP + jj < c.S).astype(np.float32)
        bmask = np.zeros((P, c.B_LOC), np.float32)
        for b in range(c.B_LOC):
            bmask[b * c.PPB : (b + 1) * c.PPB, b] = 1.0
        _CACHE[key] = (
            emb_A,
            dict(
                embI=embI,
                emb3s_all=emb3s_all,
                vmask_all=vmask_all,
                smask=smask,
                bmask=bmask,
                bmaskT=np.ascontiguousarray(bmask.T),
            ),
        )
    return _CACHE[key][1]


def _pack_story(cfg, story_c, quest_c):
    c = cfg
    sp = np.full((c.B_LOC, c.S_PAD, c.SENT), c.V, np.int32)
    sp[:, : c.S] = story_c
    sp[:, c.S] = quest_c
    return np.ascontiguousarray(sp.reshape(c.TOT, c.SENT))


def kernel(story, question, emb_A, _trace=False, _trace_kwargs=None):
    from concourse import bass_utils

    story = np.asarray(story)
    question = np.asarray(question)
    emb_A = np.asarray(emb_A, dtype=np.float32)

    cfg = Cfg(
        B=story.shape[0],
        S=story.shape[1],
        SENT=story.shape[2],
        V=emb_A.shape[1],
        K_HOP=emb_A.shape[0] - 1,
    )
    nc = _get_nc(cfg)
    shared = _pack_shared(cfg, emb_A)
    in_maps = []
    for ci in range(N_CORES):
        sl = slice(ci * cfg.B_LOC, (ci + 1) * cfg.B_LOC)
        in_maps.append(
            {
                "story_pad": _pack_story(
                    cfg, story[sl].astype(np.int32), question[sl].astype(np.int32)
                ),
                "embI": shared["embI"],
                "emb3s": shared["emb3s_all"][ci],
                "vmask": shared["vmask_all"][ci],
                "smask": shared["smask"],
                "bmask": shared["bmask"],
                "bmaskT": shared["bmaskT"],
            }
        )
    kwargs = {}
    if _trace:
        kwargs = dict(trace=True, trace_kwargs=_trace_kwargs or {})
    res = bass_utils.run_bass_kernel_spmd(
        nc, in_maps, core_ids=list(range(N_CORES)), **kwargs
    )
    full = np.concatenate(
        [r["out"].astype(np.float32).T for r in res.results], axis=1
    )[:, : cfg.V]
    outv = np.ascontiguousarray(full).astype(np.float32)
    if _trace:
        return outv, res
    return outv


# revision 14
# speedup vs baseline: 1.0116x; 1.0116x over previous
"""MemN2N Bass kernel (SPMD over 8 cores), v3: 1.18 ms (baseline 3.96 ms).

  - The 4 embedding tables are interleaved host-side into one bf16 table
    embI [V+1, 4*E]; each story token is ONE 1KB gather descriptor covering
    all 4 tables (v0: 4 separate f32 gathers -> 4x bytes, 4x instructions).
    HW ucode only honors [128, 1] indirect-DMA offset APs, so it is one
    gather instruction per token column (650 total, ~1.26 us each on Pool —
    the kernel's bottleneck); instructions alternate between 2 SWDGE queues,
    which cuts the per-instruction descriptor-generation time.
  - Per-sentence embedding-bag sums are pairwise in-place bf16 tensor_add
    trees on DVE (2x fast mode; tensor_reduce has none), overlapped with the
    gather stream.
  - The question is packed as an extra "sentence" (s = S) of every batch, so
    its embedding-bag rides the story gather for free; u0 is extracted with
    one PE transpose.
  - Attention hops run entirely on-chip (no DRAM bounces): scores come from
    a DVE multiply+reduce against a partition-replicated u, softmax
    normalization uses tiny bmask matmuls for the cross-partition sums.
  - Final vocab softmax is vocab-parallel: u3 is AllGather'd (32KB), each
    core computes logits/exp for ALL 64 batches over its 12.8k-vocab slice
    (PE: 100 [128x128]@[128x64] matmuls), the softmax denominator is
    AllReduce'd ([1,64]), and each core writes its [12800, 64] bf16 output
    slice which the host casts/transposes/concats.

Per-core HBM traffic ~90 MB (vs ~196 MB); emb3 slice prefetch overlaps hops.
"""
import sys

sys.path.insert(0, "/opt/trn_rl_repo")

from contextlib import ExitStack

import numpy as np

import concourse.bass as bass
import concourse.mybir as mybir
import concourse.tile as tile
from concourse.masks import make_identity

F32 = mybir.dt.float32
BF16 = mybir.dt.bfloat16
I32 = mybir.dt.int32
AX = mybir.AxisListType
ALU = mybir.AluOpType
ACTF = mybir.ActivationFunctionType

P = 128
E = 128
N_CORES = 8


class Cfg:
    def __init__(self, B=64, S=200, SENT=50, V=100000, K_HOP=3, TCH=25, CHT=8):
        self.B = B
        self.B_LOC = B // N_CORES  # 8
        self.S = S  # 200 real sentences
        self.SENT = SENT  # 50 tokens per sentence
        self.V = V
        self.K_HOP = K_HOP
        self.NT = K_HOP + 1  # 4 tables
        self.EI = self.NT * E  # 512 interleaved embedding width
        self.PPB = P // self.B_LOC  # 16 partitions per batch
        # sentence slots per batch padded to use all 128 partitions; S+1
        # sentences (the question rides as sentence index S)
        self.SPP = -(-(self.B_LOC * (S + 1)) // P)  # 13
        self.S_PAD = self.PPB * self.SPP  # 208
        assert self.S_PAD >= S + 1
        self.TOT = P * self.SPP  # 1664 slots
        # question slot: sentence s=S lives at (q = S//SPP, j = S%SPP)
        self.QJ = S % self.SPP
        # token gather chunking
        self.TCH = TCH  # tokens per indirect DMA
        self.NTOK = self.SPP * SENT  # idx columns per partition (650)
        assert self.NTOK % TCH == 0
        self.NCHG = self.NTOK // TCH  # 26 gather chunks
        assert SENT % TCH == 0 or TCH % SENT == 0 or SENT % TCH == 0
        self.CPS = SENT // TCH  # chunks per sentence (2)
        assert self.CPS * TCH == SENT
        # vocab-parallel final phase
        self.VPAD = -(-V // (N_CORES * P)) * (N_CORES * P)  # 102400
        self.VS = self.VPAD // N_CORES  # 12800 vocab rows per core
        self.NVT = self.VS // P  # 100 v-tiles per core
        self.CHT = CHT  # v-tiles per psum chunk
        self.NCH = -(-self.NVT // CHT)  # 13 chunks
        # [128, 1] offsets -> 128 descriptors per gather
        self.DMA_SCRATCH = 32768
        self.NQ = 4  # SWDGE queues to spread gathers over


def build_kernel(cfg: Cfg, nc: bass.Bass, dbg: bool = False):
    c = cfg
    story = nc.declare_dram_parameter("story_pad", [c.TOT, c.SENT], I32, isOutput=False)
    embI = nc.declare_dram_parameter("embI", [c.V + 1, c.EI], BF16, isOutput=False)
    emb3s = nc.declare_dram_parameter("emb3s", [E, c.VS], BF16, isOutput=False)
    vmask = nc.declare_dram_parameter("vmask", [P, c.NVT], F32, isOutput=False)
    smask = nc.declare_dram_parameter("smask", [P, c.SPP], F32, isOutput=False)
    bmask = nc.declare_dram_parameter("bmask", [P, c.B_LOC], F32, isOutput=False)
    bmaskT = nc.declare_dram_parameter("bmaskT", [c.B_LOC, P], F32, isOutput=False)
    out = nc.declare_dram_parameter("out", [c.VS, c.B], BF16, isOutput=True)

    dbgout = None
    if dbg:
        dbgout = {
            "dG0": nc.declare_dram_parameter("dG0", [P, c.EI], F32, isOutput=True),
            "duT0": nc.declare_dram_parameter("duT0", [P, c.B_LOC], F32, isOutput=True),
            "duT1": nc.declare_dram_parameter("duT1", [P, c.B_LOC], F32, isOutput=True),
            "dprobs0": nc.declare_dram_parameter("dprobs0", [P, c.SPP], F32, isOutput=True),
            "duall": nc.declare_dram_parameter("duall", [P, c.B], F32, isOutput=True),
            "dden": nc.declare_dram_parameter("dden", [1, c.B], F32, isOutput=True),
        }
    with tile.TileContext(nc) as tc:
        _body(c, nc, tc, story, embI, emb3s, vmask, smask, bmask, bmaskT, out, dbgout)
    return nc


def _body(c: Cfg, nc, tc, story, embI, emb3s, vmask, smask, bmask, bmaskT, out, dbgout=None):
    B = c.B
    BL = c.B_LOC
    with ExitStack() as es:
        cpool = es.enter_context(tc.tile_pool(name="const", bufs=1))
        upool = es.enter_context(tc.tile_pool(name="u", bufs=1))

        identity = cpool.tile([P, P], F32)
        make_identity(nc, identity[:])

        idx_t = cpool.tile([P, c.NTOK], I32)
        nc.sync.dma_start(
            out=idx_t[:], in_=story[:].rearrange("(p j) t -> p (j t)", p=P)
        )
        smask_t = cpool.tile([P, c.SPP], F32)
        nc.sync.dma_start(out=smask_t[:], in_=smask[:])
        vmask_t = cpool.tile([P, c.NVT], F32)
        nc.sync.dma_start(out=vmask_t[:], in_=vmask[:])
        bmask_t = cpool.tile([P, BL], F32)
        nc.sync.dma_start(out=bmask_t[:], in_=bmask[:])
        bmaskT_t = cpool.tile([BL, P], F32)
        nc.sync.dma_start(out=bmaskT_t[:], in_=bmaskT[:])
        ones_col = cpool.tile([P, 1], F32)
        nc.vector.memset(ones_col[:], 1.0)
        ones_row = cpool.tile([1, P], F32)
        nc.vector.memset(ones_row[:], 1.0)

        # G_all[slot, j, t*E + e] = sum over sentence tokens of interleaved tables
        G_all = cpool.tile([P, c.SPP, c.EI], F32)

        # ---------- phase A: gather + segment reduce ----------
        # HW ucode only honors [128, 1] offset APs (one gather instruction per
        # token column; 128 descriptors each). The per-sentence token sum is a
        # pairwise in-place bf16 tree: tensor_tensor add has a 2x DVE mode for
        # packed 2-byte operands (tensor_reduce has none). The last level
        # writes f32 into G_all.
        def tree_sum(buf, n):
            while n > 2:
                k = n // 2
                nc.vector.tensor_add(
                    out=buf[:, 0:k, :], in0=buf[:, 0:k, :], in1=buf[:, n - k : n, :]
                )
                n = k + (n & 1)

        with tc.tile_pool(name="gather", bufs=2) as gbpool:
            for j in range(c.SPP):
                gbuf = gbpool.tile([P, c.SENT, c.EI], BF16, tag="gbuf")
                for s in range(c.SENT):
                    gi = nc.gpsimd.indirect_dma_start(
                        out=gbuf[:, s, :],
                        out_offset=None,
                        in_=embI[:],
                        in_offset=bass.IndirectOffsetOnAxis(
                            ap=idx_t[:, j * c.SENT + s : j * c.SENT + s + 1], axis=0
                        ),
                    )
                    # spread SWDGE descriptor generation across queues
                    q = s % c.NQ
                    if q:
                        gi.ins.queue = f"qPoolDynamic{q}"
                tree_sum(gbuf, c.SENT)
                nc.vector.tensor_add(
                    out=G_all[:, j, :], in0=gbuf[:, 0, :], in1=gbuf[:, 1, :]
                )

        if dbgout is not None:
            nc.sync.dma_start(out=dbgout["dG0"][:], in_=G_all[:, 0, :])

        # ---------- final-phase weights prefetch (overlaps hops) ----------
        fpool = es.enter_context(tc.tile_pool(name="final", bufs=1))
        emb3s_t = fpool.tile([P, c.VS], BF16)
        nc.sync.dma_start(out=emb3s_t[:], in_=emb3s[:])

        # ---------- phase B: u0 from the question slot ----------
        uT_sb = None
        with tc.tile_pool(name="tp_ps", bufs=2, space="PSUM") as tppool:
            t0_ps = tppool.tile([P, P], F32, tag="t0")
            nc.tensor.matmul(
                out=t0_ps[:], lhsT=G_all[:, c.QJ, 0:E], rhs=identity[:],
                start=True, stop=True,
            )
            uT_sb = upool.tile([P, BL], F32, tag="uT0")
            # question of batch b sits at slot partition b*PPB + (PPB-1)
            nc.vector.tensor_copy(
                out=uT_sb[:],
                in_=t0_ps[:].rearrange("e (b q) -> e b q", q=c.PPB)[:, :, c.PPB - 1],
            )
            if dbgout is not None:
                nc.sync.dma_start(out=dbgout["duT0"][:], in_=uT_sb[:])

        # ---------- phase C: K_HOP attention hops, fully on-chip ----------
        with (
            tc.tile_pool(name="hop", bufs=2) as hpool,
            tc.tile_pool(name="hop_ps", bufs=1, space="PSUM") as hps,
            tc.tile_pool(name="hop_ps2", bufs=1, space="PSUM") as hps2,
        ):
            for h in range(c.K_HOP):
                # u_b [BL, E] = uT^T
                ub_ps = hps.tile([BL, P], F32, tag="ub")
                nc.tensor.matmul(
                    out=ub_ps[:], lhsT=uT_sb[:], rhs=identity[:], start=True, stop=True
                )
                ub_sb = hpool.tile([BL, P], F32, tag="ub_sb")
                nc.vector.tensor_copy(out=ub_sb[:], in_=ub_ps[:])
                # urep [p, e] = u[batch(p), e]
                urep_ps = hps.tile([P, P], F32, tag="urep")
                nc.tensor.matmul(
                    out=urep_ps[:], lhsT=bmaskT_t[:], rhs=ub_sb[:], start=True, stop=True
                )
                urep_sb = hpool.tile([P, P], F32, tag="urep_sb")
                nc.vector.tensor_copy(out=urep_sb[:], in_=urep_ps[:])
                # scores[p, j] = sum_e G_m[p, j, e] * urep[p, e]
                sc_tmp = hpool.tile([P, c.SPP, E], F32, tag="sc_tmp")
                nc.vector.tensor_tensor(
                    out=sc_tmp[:],
                    in0=G_all[:, :, h * E : (h + 1) * E],
                    in1=urep_sb[:].unsqueeze(1).to_broadcast([P, c.SPP, E]),
                    op=ALU.mult,
                )
                scores = hpool.tile([P, c.SPP], F32, tag="scores")
                nc.vector.tensor_reduce(
                    out=scores[:].unsqueeze(-1), in_=sc_tmp[:], axis=AX.X, op=ALU.add
                )
                # masked exp (scores are O(10); no max-subtraction needed in f32)
                exps = hpool.tile([P, c.SPP], F32, tag="exps")
                nc.scalar.activation(out=exps[:], in_=scores[:], func=ACTF.Exp)
                nc.vector.tensor_tensor(
                    out=exps[:], in0=exps[:], in1=smask_t[:], op=ALU.mult
                )
                rowsum = hpool.tile([P, 1], F32, tag="rowsum")
                nc.vector.tensor_reduce(
                    out=rowsum[:], in_=exps[:], axis=AX.X, op=ALU.add
                )
                den_ps = hps2.tile([BL, 1], F32, tag="den")
                nc.tensor.matmul(
                    out=den_ps[:], lhsT=bmask_t[:], rhs=rowsum[:], start=True, stop=True
                )
                rec8 = hpool.tile([BL, 1], F32, tag="rec8")
                nc.vector.reciprocal(out=rec8[:], in_=den_ps[:])
                rep_ps = hps2.tile([P, 1], F32, tag="rep")
                nc.tensor.matmul(
                    out=rep_ps[:], lhsT=bmaskT_t[:], rhs=rec8[:], start=True, stop=True
                )
                recrep = hpool.tile([P, 1], F32, tag="recrep")
                nc.vector.tensor_copy(out=recrep[:], in_=rep_ps[:])
                probs = hpool.tile([P, c.SPP], F32, tag="probs")
                nc.vector.tensor_scalar_mul(probs[:], exps[:], recrep[:])
                if dbgout is not None and h == 0:
                    nc.sync.dma_start(out=dbgout["dprobs0"][:], in_=probs[:])
                # block-diagonal weights and combine
                bd = hpool.tile([P, c.SPP, BL], F32, tag="bd")
                nc.vector.tensor_tensor(
                    out=bd[:],
                    in0=probs[:].unsqueeze(-1).to_broadcast([P, c.SPP, BL]),
                    in1=bmask_t[:].unsqueeze(1).to_broadcast([P, c.SPP, BL]),
                    op=ALU.mult,
                )
                uc_ps = hps.tile([P, BL], F32, tag="uc")
                for j in range(c.SPP):
                    nc.tensor.matmul(
                        out=uc_ps[:],
                        lhsT=G_all[:, j, (h + 1) * E : (h + 2) * E],
                        rhs=bd[:, j, :],
                        start=(j == 0),
                        stop=(j == c.SPP - 1),
                    )
                uT_new = upool.tile([P, BL], F32, tag=f"uT{h + 1}")
                nc.vector.tensor_add(out=uT_new[:], in0=uc_ps[:], in1=uT_sb[:])
                uT_sb = uT_new
                if dbgout is not None and h == 0:
                    nc.sync.dma_start(out=dbgout["duT1"][:], in_=uT_sb[:])

        # ---------- phase D: vocab-parallel logits + softmax ----------
        with (
            tc.tile_pool(name="fin_ps", bufs=2, space="PSUM") as fps,
            tc.tile_pool(name="fin_ps2", bufs=1, space="PSUM") as fps2,
            tc.tile_pool(name="dram", bufs=1, space="DRAM") as dram,
        ):
            # AllGather u3 across the 8 cores
            u3b_ps = fps2.tile([BL, P], F32, tag="u3b")
            nc.tensor.matmul(
                out=u3b_ps[:], lhsT=uT_sb[:], rhs=identity[:], start=True, stop=True
            )
            u3b_sb = fpool.tile([BL, P], F32)
            nc.vector.tensor_copy(out=u3b_sb[:], in_=u3b_ps[:])
            uin_d = dram.tile([BL, P], F32, name="uin_d")
            nc.gpsimd.dma_start(out=uin_d[:], in_=u3b_sb[:])
            uall_d = dram.tile([B, P], F32, name="uall_d")
            nc.gpsimd.collective_compute(
                "AllGather",
                ALU.bypass,
                replica_groups=[list(range(N_CORES))],
                ins=[uin_d.opt()],
                outs=[uall_d.opt()],
            )
            uall_sb = fpool.tile([B, P], F32)
            nc.gpsimd.dma_start(out=uall_sb[:], in_=uall_d[:])
            uTall_ps = fps2.tile([P, B], F32, tag="uTall")
            nc.tensor.matmul(
                out=uTall_ps[:], lhsT=uall_sb[:], rhs=identity[:B, :B],
                start=True, stop=True,
            )
            uTall_bf = fpool.tile([P, B], BF16)
            nc.vector.tensor_copy(out=uTall_bf[:], in_=uTall_ps[:])
            if dbgout is not None:
                nc.sync.dma_start(out=dbgout["duall"][:], in_=uTall_ps[:])

            # logits + exp over this core's vocab slice, all 64 batches
            exp_buf = fpool.tile([P, c.NVT * B], F32)
            for ch in range(c.NCH):
                t0 = ch * c.CHT
                nt = min(c.CHT, c.NVT - t0)
                lg = fps.tile([P, c.CHT * B], F32, tag="lg")
                for m in range(nt):
                    nc.tensor.matmul(
                        out=lg[:, m * B : (m + 1) * B],
                        lhsT=emb3s_t[:, (t0 + m) * P : (t0 + m + 1) * P],
                        rhs=uTall_bf[:],
                        start=True,
                        stop=True,
                    )
                nc.scalar.activation(
                    out=exp_buf[:, t0 * B : t0 * B + nt * B],
                    in_=lg[:, : nt * B],
                    func=ACTF.Exp,
                )
            exp3 = exp_buf[:].rearrange("p (m b) -> p m b", b=B)
            # zero out vocab-pad rows
            nc.vector.tensor_tensor(
                out=exp3,
                in0=exp3,
                in1=vmask_t[:].unsqueeze(-1).to_broadcast([P, c.NVT, B]),
                op=ALU.mult,
            )
            # local denominator: reduce over tiles, then over partitions
            part_den = fpool.tile([P, B], F32)
            nc.vector.tensor_reduce(
                out=part_den[:].unsqueeze(-1),
                in_=exp_buf[:].rearrange("p (m b) -> p b m", b=B),
                axis=AX.X,
                op=ALU.add,
            )
            dsum_ps = fps2.tile([1, B], F32, tag="dsum")
            nc.tensor.matmul(
                out=dsum_ps[:], lhsT=ones_col[:], rhs=part_den[:], start=True, stop=True
            )
            den_sb = fpool.tile([1, B], F32)
            nc.vector.tensor_copy(out=den_sb[:], in_=dsum_ps[:])
            den_i = dram.tile([1, B], F32, name="den_i")
            nc.gpsimd.dma_start(out=den_i[:], in_=den_sb[:])
            den_o = dram.tile([1, B], F32, name="den_o")
            nc.gpsimd.collective_compute(
                "AllReduce",
                ALU.add,
                replica_groups=[list(range(N_CORES))],
                ins=[den_i.opt()],
                outs=[den_o.opt()],
            )
            dent_sb = fpool.tile([1, B], F32)
            nc.gpsimd.dma_start(out=dent_sb[:], in_=den_o[:])
            if dbgout is not None:
                nc.sync.dma_start(out=dbgout["dden"][:], in_=dent_sb[:])
            rec1 = fpool.tile([1, B], F32)
            nc.vector.reciprocal(out=rec1[:], in_=dent_sb[:])
            rep_ps = fps2.tile([P, B], F32, tag="rrep")
            nc.tensor.matmul(
                out=rep_ps[:], lhsT=ones_row[:], rhs=rec1[:], start=True, stop=True
            )
            rec_sb = fpool.tile([P, B], F32)
            nc.vector.tensor_copy(out=rec_sb[:], in_=rep_ps[:])
            prob_bf = fpool.tile([P, c.NVT * B], BF16)
            nc.vector.tensor_tensor(
                out=prob_bf[:].rearrange("p (m b) -> p m b", b=B),
                in0=exp3,
                in1=rec_sb[:].unsqueeze(1).to_broadcast([P, c.NVT, B]),
                op=ALU.mult,
            )
            # out[v = m*128 + p, b] = probs
            nc.sync.dma_start(
                out=out[:].rearrange("(m p) b -> p m b", p=P),
                in_=prob_bf[:].rearrange("p (m b) -> p m b", b=B),
            )


# ---------------- host-side pack/unpack ----------------
_CACHE = {}


def _get_nc(cfg):
    key = "nc"
    if key not in _CACHE:
        import concourse.bacc as bacc

        nc = bacc.Bacc(
            target_bir_lowering=False,
            dynamic_dma_scratch_size=cfg.DMA_SCRATCH,
            num_swdge_queues=cfg.NQ,
        )
        build_kernel(cfg, nc)
        nc.finalize()
        _CACHE[key] = nc
    return _CACHE[key]


def _pack_shared(cfg, emb_A):
    key = "shared"
    if key not in _CACHE or _CACHE[key][0] is not emb_A:
        c = cfg
        import ml_dtypes

        NT, V, _ = emb_A.shape
        # interleaved bf16 table, zero pad row at V
        embI = np.zeros((V + 1, c.EI), ml_dtypes.bfloat16)
        embI[:V] = emb_A.transpose(1, 0, 2).reshape(V, c.EI)
        # per-core transposed slices of the last table
        e3T = np.zeros((E, c.VPAD), np.float32)
        e3T[:, :V] = emb_A[NT - 1].T
        e3T = e3T.astype(ml_dtypes.bfloat16)
        emb3s_all = [
            np.ascontiguousarray(e3T[:, ci * c.VS : (ci + 1) * c.VS])
            for ci in range(N_CORES)
        ]
        vmask_all = []
        for ci in range(N_CORES):
            vrow = ci * c.VS + np.arange(c.NVT)[None, :] * P + np.arange(P)[:, None]
            vmask_all.append((vrow < V).astype(np.float32))
        pj = np.arange(P)[:, None] % c.PPB
        jj = np.arange(c.SPP)[None, :]
        smask = (pj * c.SPP + jj < c.S).astype(np.float32)
        bmask = np.zeros((P, c.B_LOC), np.float32)
        for b in range(c.B_LOC):
            bmask[b * c.PPB : (b + 1) * c.PPB, b] = 1.0
        _CACHE[key] = (
            emb_A,
            dict(
                embI=embI,
                emb3s_all=emb3s_all,
                vmask_all=vmask_all,
                smask=smask,
                bmask=bmask,
                bmaskT=np.ascontiguousarray(bmask.T),
            ),
        )
    return _CACHE[key][1]


def _pack_story(cfg, story_c, quest_c):
    c = cfg
    sp = np.full((c.B_LOC, c.S_PAD, c.SENT), c.V, np.int32)
    sp[:, : c.S] = story_c
    sp[:, c.S] = quest_c
    return np.ascontiguousarray(sp.reshape(c.TOT, c.SENT))


def kernel(story, question, emb_A, _trace=False, _trace_kwargs=None):
    from concourse import bass_utils

    story = np.asarray(story)
    question = np.asarray(question)
    emb_A = np.asarray(emb_A, dtype=np.float32)

    cfg = Cfg(
        B=story.shape[0],
        S=story.shape[1],
        SENT=story.shape[2],
        V=emb_A.shape[1],
        K_HOP=emb_A.shape[0] - 1,
    )
    nc = _get_nc(cfg)
    shared = _pack_shared(cfg, emb_A)
    in_maps = []
    for ci in range(N_CORES):
        sl = slice(ci * cfg.B_LOC, (ci + 1) * cfg.B_LOC)
        in_maps.append(
            {
                "story_pad": _pack_story(
                    cfg, story[sl].astype(np.int32), question[sl].astype(np.int32)
                ),
                "embI": shared["embI"],
                "emb3s": shared["emb3s_all"][ci],
                "vmask": shared["vmask_all"][ci],
                "smask": shared["smask"],
                "bmask": shared["bmask"],
                "bmaskT": shared["bmaskT"],
            }
        )
    kwargs = {}
    if _trace:
        kwargs = dict(trace=True, trace_kwargs=_trace_kwargs or {})
    res = bass_utils.run_bass_kernel_spmd(
        nc, in_maps, core_ids=list(range(N_CORES)), **kwargs
    )
    full = np.concatenate(
        [r["out"].astype(np.float32).T for r in res.results], axis=1
    )[:, : cfg.V]
    outv = np.ascontiguousarray(full).astype(np.float32)
    if _trace:
        return outv, res
    return outv
